# revision 31
# baseline (speedup 1.0000x reference)
"""Trainium2 Bass kernel: LSTM neighbor-sequence aggregator + projection.

Model (reference): for each node v, run an LSTM (H=256) over the features
(F=128) of the targets of v's outgoing edges (in original edge order), take
the hidden state at the last valid step, concat with v's own features, and
project with W_out ([F+H, OUT]).

Strategy
--------
Host (numpy):
  * Edges sorted by src (stable) -> per-node neighbor id lists.
  * Nodes dealt round-robin by global degree rank onto 8 cores, so per-core
    degree profiles match within +-1 at every step.
  * A shared step schedule M_t (non-increasing) is built so that on every
    core the set of columns active at LSTM step t is exactly [0, M_t).
    Each node is placed at a column whose "lifetime" equals its degree;
    leftover columns are dummies (zero inputs, results discarded).
  * Neighbor features are pre-gathered into a step-major packed stream
    xseq [F=128, S] per core (S = sum_t M_t ~ E/8 * 1.04), bf16.

Device (Bass/Tile, identical program on 8 cores):
  * The kernel is ACT(scalar-engine)-bound: 8 gate elems + 2 tanh(c) elems
    per column-step at 1 elem/cycle/lane.  Columns are processed in
    1024-wide superchunks; gate passes (i, g, f, o) of 2 blocks each cycle
    through two 4-bank PSUM tiles.
  * Per bank the PE lands three matmuls: a K=8 delta-pattern matmul that
    writes the gate bias everywhere (start=True), W_ih @ x (bf16), and
    W_hh @ h as ONE fp8 DoubleRow matmul (K=256 via 2 k-tiles, 2x vs a
    bf16 pair; h is stored fp8e4 [128, 2, w] which is exactly the
    DoubleRow rhs layout).  With bias pre-added, a full pass drains with a
    single merged ACT over all 4 banks — PE and ACT run ~balanced, which
    also keeps the PE HAM window busy (no half-clock throttling).
  * DVE: c = f*c + i*g (c fp32), h = o*tanh(c) (bf16 inputs, fp8 h out).
    Columns taking their final step also write a bf16 h copy, so the
    projection reads full-precision h.
  * Deep tail (M_t <= 128): latency-bound; bf16 weights/x/h, 4-8 gate
    blocks packed per PSUM bank + 3 grouped ACTs; filler matmuls keep HAM
    at full clock.
  * Final projection per 512-chunk: out[256, w] = W_out.T @ [x_own; h]
    (x f32r + h bf16), spread through the tail as PE filler.
"""

import math
import os
import sys

for _p in (
    "/opt/trn_rl_repo",
    "/root/.axon_site",
    "/root/.axon_site/_ro/trn_rl_repo",
    "/root/.axon_site/_ro/pypackages",
):
    if os.path.isdir(_p) and _p not in sys.path:
        sys.path.append(_p)

import numpy as np

import concourse.bass as bass
import concourse.tile as tile
from concourse import bacc, mybir
from concourse.bass_utils import run_bass_kernel_spmd

NCORES = 8
F, H, OUT = 128, 256, 256
CH = 512    # projection chunk width / PSUM bank width (fp32)
SC = 1024   # superchunk width (merged pass-ACT span, 4 PSUM banks)
DEEP_W = 128  # switch to packed deep-tail regime when M_t <= DEEP_W

F32 = mybir.dt.float32
F32R = mybir.dt.float32r
BF16 = mybir.dt.bfloat16
FP8 = mybir.dt.float8e4
DR = mybir.MatmulPerfMode.DoubleRow

_SIG = mybir.ActivationFunctionType.Sigmoid
_TANH = mybir.ActivationFunctionType.Tanh
# gate blocks (PyTorch order i,f,g,o; two 128-row blocks each)
# pass order: i (blocks 0,1), g (4,5), f (2,3), o (6,7)
_PASSES = [(0, 1, _SIG), (4, 5, _TANH), (2, 3, _SIG), (6, 7, _SIG)]


# ---------------------------------------------------------------- host side

def _preprocess(input_matrix, adjacency):
    """Partition nodes, build shared schedule + packed per-core inputs."""
    N = input_matrix.shape[0]
    src, trg = adjacency[0], adjacency[1]

    order = np.argsort(src, kind="stable")
    trg_s = trg[order]
    counts = np.bincount(src, minlength=N).astype(np.int64)
    offsets = np.zeros(N + 1, np.int64)
    np.cumsum(counts, out=offsets[1:])

    rank_order = np.argsort(-counts, kind="stable")
    core_nodes = [rank_order[c::NCORES] for c in range(NCORES)]
    deg_c = [counts[cn] for cn in core_nodes]

    T = int(counts.max())
    cnt = np.zeros((NCORES, T + 1), np.int64)
    for c in range(NCORES):
        h = np.bincount(deg_c[c], minlength=T + 1)
        cs = np.cumsum(h)
        cnt[c, :] = len(deg_c[c]) - cs[: T + 1]
    D = np.max(cnt[:, :-1] - cnt[:, 1:], axis=0)  # D[d-1] for d=1..T
    # round every level up to a multiple of 2 (even widths for DVE 2x
    # mode) while preserving capacity
    M = np.zeros(T + 1, np.int64)
    for t in range(T - 1, -1, -1):
        M[t] = -(-(M[t + 1] + D[t]) // 2) * 2

    ALL_COL = int(M[0])
    col_node = []
    deg0 = []
    for c in range(NCORES):
        cn = np.full(ALL_COL, -1, np.int64)
        for d in range(T, 0, -1):
            s0 = int(cnt[c, d])
            k = int(cnt[c, d - 1]) - s0
            if k:
                cn[int(M[d]) : int(M[d]) + k] = core_nodes[c][s0 : s0 + k]
        deg0.append(core_nodes[c][deg_c[c] == 0])  # handled on host
        col_node.append(cn)

    Mt = M[:-1]
    off = np.zeros(T + 1, np.int64)
    np.cumsum(Mt, out=off[1:])
    S = int(off[T])

    xseq = []
    xown = []
    im32 = np.ascontiguousarray(input_matrix, np.float32)
    for c in range(NCORES):
        xs = np.zeros((S, F), np.float32)
        cn = col_node[c]
        for t in range(T):
            m = int(Mt[t])
            colnodes = cn[:m]
            valid = colnodes >= 0
            vnodes = colnodes[valid]
            nbr = trg_s[offsets[vnodes] + t]
            xs[off[t] : off[t] + m][valid] = im32[nbr]
        xseq.append(np.ascontiguousarray(xs.T))
        xo = np.zeros((ALL_COL, F), np.float32)
        valid = cn >= 0
        xo[valid] = im32[cn[valid]]
        xown.append(np.ascontiguousarray(xo.T))

    return dict(T=T, M=Mt, off=off, S=S, AC=ALL_COL, xseq=xseq, xown=xown,
                col_node=col_node, deg0=deg0)


# ------------------------------------------------------------- bass program

def build_program(T, Mt, off, S, AC, use_dr=True):
    """One SPMD program shared by all cores (schedule baked in)."""
    nc = bacc.Bacc("TRN2", target_bir_lowering=False, debug=False,
                   enable_asserts=False)

    # superchunk geometry: widths padded so the fp8 k-tile stride (in bytes)
    # is a multiple of 16 (DoubleRow AP requirement)
    NSC = math.ceil(AC / SC)
    SCO = [j * SC for j in range(NSC)]
    SCW = [min(SC, AC - j * SC) for j in range(NSC)]
    SCWp = [-(-w // 16) * 16 for w in SCW]

    TSW2 = next((t for t in range(1, T) if Mt[t] <= DEEP_W), T)

    h_dt = FP8 if use_dr else BF16

    xseq_d = nc.declare_dram_parameter("xseq", [128, S], BF16, isOutput=False)
    xown_d = nc.declare_dram_parameter("xown", [128, AC], F32R, isOutput=False)
    # bf16 weights: [0]=W_ih.T, [1]=W_hh.T[:128], [2]=W_hh.T[128:]
    wl_d = nc.declare_dram_parameter("wl", [3, 128, 1024], BF16, isOutput=False)
    whh8_d = nc.declare_dram_parameter("whh8", [128, 8, 2, 128], FP8,
                                       isOutput=False)
    wox_d = nc.declare_dram_parameter("wox", [128, 256], F32R, isOutput=False)
    wohb_d = nc.declare_dram_parameter("wohb", [2, 128, 256], BF16,
                                       isOutput=False)
    bc_d = nc.declare_dram_parameter("bc", [128, 8], F32, isOutput=False)
    bct8_d = nc.declare_dram_parameter("bct8", [8, 128], BF16, isOutput=False)
    # bcrep[k, b, m] = bias[b*128+m] / 128, same for all k: the bias matmul
    # bcrep[:, b, :].T @ ones writes bias block b across a full PSUM bank
    # using the full 128-row PE array (small-K matmuls keep HAM throttled)
    bcrep_d = nc.declare_dram_parameter("bcrep", [128, 8, 128], BF16,
                                        isOutput=False)
    ones_d = nc.declare_dram_parameter("ones", [128, 512], BF16,
                                       isOutput=False)
    be8_d = nc.declare_dram_parameter("be8", [8, 8, 64], BF16, isOutput=False)
    be4_d = nc.declare_dram_parameter("be4", [4, 4, 128], BF16, isOutput=False)
    out_d = nc.declare_dram_parameter("out", [2, 128, AC], F32, isOutput=True)

    NCH = math.ceil(AC / CH)
    last_touch = [max(t for t in range(T) if Mt[t] > j * CH) for j in range(NCH)]
    # chunk 0 projects at the very end (from h_b); the rest are spread over
    # the narrow-step region as PE filler work
    proj_at = [T - 1] + [min(max(last_touch[j] + 1, TSW2 - 10 + (NCH - 1 - j)),
                             T - 2)
                         for j in range(1, NCH)]

    with tile.TileContext(nc) as tc:
        with (
            tc.tile_pool(name="const", bufs=1) as constp,
            tc.tile_pool(name="state", bufs=1) as statep,
            tc.tile_pool(name="xin", bufs=4) as xinp,
            tc.tile_pool(name="gates", bufs=2) as gatep,
            tc.tile_pool(name="tmp", bufs=3) as tmpp,
            tc.tile_pool(name="outs", bufs=3) as outsp,
        ):
            # weights via the gpsimd DMA queue so xseq loads (sync queue)
            # aren't stuck behind them
            wih = constp.tile([128, 1024], BF16, tag="wih")
            bias = constp.tile([128, 8], F32, tag="bias")
            scr = constp.tile([128, 1], F32, tag="scr")
            nc.gpsimd.dma_start(wih[:], wl_d[0])
            nc.gpsimd.dma_start(bias[:], bc_d[:])
            # dummy 1-elem sigmoid pulls the ACT table load into the startup
            # DMA window
            nc.scalar.activation(scr[:, 0:1], bias[:, 0:1], _SIG)
            bct8 = constp.tile([8, 128], BF16, tag="bct8")
            bcrep = constp.tile([128, 8, 128], BF16, tag="bcrep")
            ones = constp.tile([128, 512], BF16, tag="ones")
            nc.gpsimd.dma_start(bct8[:], bct8_d[:])
            nc.gpsimd.dma_start(bcrep[:], bcrep_d[:])
            nc.gpsimd.dma_start(ones[:], ones_d[:])
            whh8 = constp.tile([128, 8, 2, 128], FP8, tag="whh8")
            nc.gpsimd.dma_start(whh8[:], whh8_d[:])
            w_h0_b = constp.tile([128, 1024], BF16, tag="wh0b")
            w_h1_b = constp.tile([128, 1024], BF16, tag="wh1b")
            nc.gpsimd.dma_start(w_h0_b[:], wl_d[1])
            nc.gpsimd.dma_start(w_h1_b[:], wl_d[2])
            wox = constp.tile([128, 256], F32R, tag="wox")
            nc.gpsimd.dma_start(wox[:], wox_d[:])
            w_o_b = []
            for k in range(2):
                t_ = constp.tile([128, 256], BF16, tag=f"wob{k}")
                nc.gpsimd.dma_start(t_[:], wohb_d[k])
                w_o_b.append(t_)
            h_b = constp.tile([128, 2, CH], BF16, tag="hb")
            bct4b = constp.tile([4, 128], BF16, tag="bct4b")
            be8 = constp.tile([8, 8, 64], BF16, tag="be8")
            be4 = constp.tile([4, 4, 128], BF16, tag="be4")
            nc.gpsimd.dma_start(bct4b[:], bct8_d[4:8])
            nc.gpsimd.dma_start(be8[:], be8_d[:])
            nc.gpsimd.dma_start(be4[:], be4_d[:])

            # per-superchunk recurrent state; step 0 (M_0 == AC) writes every
            # column before anything reads it.  h16 holds the bf16 copy of h
            # written only at each column's final step (projection input).
            h_t, c_t, h16_t = [], [], []
            for j in range(NSC):
                ht = statep.tile([128, 2, SCWp[j]], h_dt, tag=f"h{j}")
                ct = statep.tile([128, 2, SCWp[j]], F32, tag=f"c{j}")
                g16 = statep.tile([128, 2, SCWp[j]], BF16, tag=f"h16_{j}")
                h_t.append(ht)
                c_t.append(ct)
                h16_t.append(g16)

            def wx_sl(mi):
                return wih[:, mi * 128 : (mi + 1) * 128]

            def emit_h_writes(t, j, ga, w, G, th):
                """h = sig(o)*tanh(c): fp8 for the recurrence; columns taking
                their final step also get a bf16 copy for the projection."""
                nc.vector.tensor_mul(h_t[j][:, :, ga : ga + w], G[:, 6:8, :w],
                                     th[:, :, :w])
                fin_lo = int(Mt[t + 1]) if t + 1 < T else 0
                fin_hi = int(Mt[t])
                lo = max(fin_lo - SCO[j], ga)
                hi = min(fin_hi - SCO[j], ga + w)
                if hi > lo:
                    gl, gh = lo - ga, hi - ga
                    nc.vector.tensor_mul(h16_t[j][:, :, lo:hi],
                                         G[:, 6:8, gl:gh], th[:, :, gl:gh])

            def emit_proj(j, ps_alloc):
                """out[o, col] = W_out.T @ [x_own; h] for finished chunk j."""
                j0 = j * CH
                w = min(CH, AC - j0)
                jj, jo = j0 // SC, j0 % SC
                xo = xinp.tile([128, CH], F32R, tag="xo")
                nc.sync.dma_start(xo[:, :w], xown_d[:, j0 : j0 + w])
                use_b = j == 0 and TSW2 < T  # chunk 0 h lives in h_b
                ph = h_b if use_b else h16_t[jj]
                o0 = 0 if use_b else jo
                for mb in range(2):
                    ps = ps_alloc()
                    sl = slice(mb * 128, (mb + 1) * 128)
                    nc.tensor.matmul(ps[:, :w], wox[:, sl],
                                     xo[:, :w], start=True, stop=False)
                    nc.tensor.matmul(ps[:, :w], w_o_b[0][:, sl],
                                     ph[:, 0, o0 : o0 + w],
                                     start=False, stop=False)
                    nc.tensor.matmul(ps[:, :w], w_o_b[1][:, sl],
                                     ph[:, 1, o0 : o0 + w],
                                     start=False, stop=True)
                    ot = outsp.tile([128, CH], F32, tag="ot")
                    nc.vector.tensor_copy(ot[:, :w], ps[:, :w])
                    nc.sync.dma_start(out_d[mb, :, j0 : j0 + w], ot[:, :w])

            # ---------------- wide regime: superchunk pass structure
            with tc.tile_pool(name="psumW", bufs=2, space="PSUM") as psumw:
                for t in range(min(TSW2, T)):
                    m = int(Mt[t])
                    # segments (sc index, local col offset, width): narrow
                    # steps split in two so consecutive steps' latency
                    # chains overlap
                    if m > SC:
                        segs = []
                        for j in range(NSC):
                            if SCO[j] >= m:
                                break
                            segs.append((j, 0, min(SCW[j], m - SCO[j])))
                    elif m > 256:
                        w1 = (m // 2 + 15) // 16 * 16
                        segs = [(0, 0, w1), (0, w1, m - w1)]
                    else:
                        segs = [(0, 0, m)]
                    for (j, ga, w) in segs:
                        wh = [min(CH, w), max(0, w - CH)]
                        xt = xinp.tile([128, SC], BF16, tag="x")
                        o_t = int(off[t]) + SCO[j] + ga
                        nc.sync.dma_start(xt[:, :w], xseq_d[:, o_t : o_t + w])
                        G = gatep.tile([128, 8, SC], BF16, tag="G")
                        for pi, (bA, bB, fn) in enumerate(_PASSES):
                            if t == 0 and fn is _SIG and bA == 2:
                                continue  # f gate unused at step 0 (c0 = 0)
                            P = psumw.tile([128, 2, 2, CH], F32, tag="ps4")
                            for bi, b in enumerate((bA, bB)):
                                for hc in (0, 1):
                                    wc = wh[hc]
                                    if wc == 0:
                                        continue
                                    a = hc * CH
                                    ha = ga + a
                                    # bias lands first (start=True clears the
                                    # bank); gate matmuls accumulate onto it
                                    nc.tensor.matmul(
                                        P[:, bi, hc, :wc], bcrep[:, b, :],
                                        ones[:, :wc], start=True,
                                        stop=False,
                                        skip_group_check=True)
                                    nc.tensor.matmul(
                                        P[:, bi, hc, :wc], wx_sl(b),
                                        xt[:, a : a + wc],
                                        start=False, stop=(t == 0),
                                        skip_group_check=True)
                                    if t > 0:
                                        if use_dr:
                                            nc.tensor.matmul(
                                                P[:, bi, hc, :wc], whh8[:, b],
                                                h_t[j][:, :, ha : ha + wc],
                                                perf_mode=DR,
                                                start=False, stop=True,
                                                skip_group_check=True)
                                        else:
                                            nc.tensor.matmul(
                                                P[:, bi, hc, :wc],
                                                w_h0_b[:, b * 128 : b * 128 + 128],
                                                h_t[j][:, 0, ha : ha + wc],
                                                start=False, stop=False,
                                                skip_group_check=True)
                                            nc.tensor.matmul(
                                                P[:, bi, hc, :wc],
                                                w_h1_b[:, b * 128 : b * 128 + 128],
                                                h_t[j][:, 1, ha : ha + wc],
                                                start=False, stop=True,
                                                skip_group_check=True)
                            # pass ACT(s), bias already in PSUM.  Full halves
                            # merge into one [2 blocks, 2 halves, 512] ACT;
                            # a partial second half gets its own exact-size
                            # piece unless junk-padding is cheaper.
                            if wh[1] in (0, CH) or wh[1] > 336:
                                we = CH if wh[1] else wh[0]
                                nh = 2 if wh[1] else 1
                                src = (P[:, :, :, :] if nh == 2
                                       else P[:, :, 0, :we])
                                dst = G[:, bA : bA + 2, : nh * we]
                                if nh == 2:
                                    dst = dst.rearrange(
                                        "p b (h c) -> p b h c", h=2)
                                nc.scalar.activation(dst, src, fn)
                            else:
                                nc.scalar.activation(
                                    G[:, bA : bA + 2, :CH], P[:, :, 0, :], fn)
                                nc.scalar.activation(
                                    G[:, bA : bA + 2, CH : CH + wh[1]],
                                    P[:, :, 1, : wh[1]], fn)
                            if pi == 1:
                                # after i and g passes: t1 = sig(i)*tanh(g)
                                if t == 0:
                                    nc.vector.tensor_mul(
                                        c_t[j][:, :, ga : ga + w],
                                        G[:, 0:2, :w], G[:, 4:6, :w])
                                else:
                                    t1 = tmpp.tile([128, 2, SC], BF16, tag="t1")
                                    nc.vector.tensor_mul(
                                        t1[:, :, :w], G[:, 0:2, :w],
                                        G[:, 4:6, :w])
                            elif pi == 2:
                                cv = c_t[j][:, :, ga : ga + w]
                                nc.vector.tensor_mul(cv, cv, G[:, 2:4, :w])
                                nc.vector.tensor_add(cv, cv, t1[:, :, :w])
                        th = tmpp.tile([128, 2, SC], BF16, tag="th")
                        nc.scalar.activation(th[:, :, :w],
                                             c_t[j][:, :, ga : ga + w], _TANH)
                        emit_h_writes(t, j, ga, w, G, th)
                    # projections of finished chunks fill the PE during
                    # ACT-bound narrow steps
                    def _alloc_w():
                        psj = psumw.tile([128, 2, 2, CH], F32, tag="ps4",
                                         name="psj")
                        return psj[:, 0, 0, :]
                    for jc in range(NCH):
                        if proj_at[jc] == t:
                            emit_proj(jc, _alloc_w)

            # ---------------- deep regime: packed banks, bf16
            with (
                tc.tile_pool(name="psumD", bufs=2, space="PSUM") as psumd,
                tc.tile_pool(name="psumF", bufs=2, space="PSUM") as psumf,
                tc.tile_pool(name="psumP", bufs=2, space="PSUM") as psump,
            ):
                if TSW2 < T:
                    # snapshot chunk-0 h into the bf16 tail copy: active
                    # columns from the fp8 recurrent state, finished columns
                    # from the bf16 final-h copy
                    wa = int(Mt[TSW2])
                    nc.vector.tensor_copy(h_b[:, :, :wa], h_t[0][:, :, :wa])
                    wc = min(CH, AC)
                    nc.vector.tensor_copy(h_b[:, :, wa:wc],
                                          h16_t[0][:, :, wa:wc])
                for t in range(TSW2, T):
                    m = int(Mt[t])
                    w = m
                    xt = xinp.tile([128, SC], BF16, tag="x")
                    o_t = int(off[t])
                    nc.sync.dma_start(xt[:, :w], xseq_d[:, o_t : o_t + w])
                    G = gatep.tile([128, 8, SC], BF16, tag="G")
                    nb = 1 if w <= 64 else 2          # banks
                    bpb = 8 // nb                     # gate blocks/bank
                    be = be8 if nb == 1 else be4
                    ps = psumd.tile([128, 2, CH], F32, tag="psD")
                    psv = []
                    for bk in range(nb):
                        pv = ps[:, bk, :].rearrange("p (k c) -> p k c", k=bpb)
                        psv.append(pv)
                        blt = bct8[0:bpb, :] if bk == 0 else bct4b[:]
                        nc.tensor.matmul(ps[:, bk, :], blt, be[:, :, :],
                                         start=True, stop=False,
                                         skip_group_check=True)
                        for k in range(bpb):
                            mi = bk * bpb + k
                            sl = slice(mi * 128, (mi + 1) * 128)
                            last = k == bpb - 1
                            nc.tensor.matmul(pv[:, k, :w], wx_sl(mi),
                                             xt[:, :w], start=False,
                                             stop=False,
                                             skip_group_check=True)
                            nc.tensor.matmul(pv[:, k, :w], w_h0_b[:, sl],
                                             h_b[:, 0, :w], start=False,
                                             stop=False,
                                             skip_group_check=True)
                            nc.tensor.matmul(pv[:, k, :w], w_h1_b[:, sl],
                                             h_b[:, 1, :w], start=False,
                                             stop=last,
                                             skip_group_check=True)
                    if nb == 1:
                        pv = psv[0]
                        nc.scalar.activation(G[:, 0:4, :w], pv[:, 0:4, :w], _SIG)
                        nc.scalar.activation(G[:, 4:6, :w], pv[:, 4:6, :w], _TANH)
                        nc.scalar.activation(G[:, 6:8, :w], pv[:, 6:8, :w], _SIG)
                    else:
                        nc.scalar.activation(G[:, 0:4, :w], psv[0][:, :, :w], _SIG)
                        nc.scalar.activation(G[:, 4:6, :w], psv[1][:, 0:2, :w], _TANH)
                        nc.scalar.activation(G[:, 6:8, :w], psv[1][:, 2:4, :w], _SIG)

                    cv = c_t[0][:, :, :w]
                    t1 = tmpp.tile([128, 2, SC], BF16, tag="t1")
                    th = tmpp.tile([128, 2, SC], BF16, tag="th")
                    nc.vector.tensor_mul(t1[:, :, :w], G[:, 0:2, :w],
                                         G[:, 4:6, :w])
                    nc.vector.tensor_mul(cv, cv, G[:, 2:4, :w])
                    nc.vector.tensor_add(cv, cv, t1[:, :, :w])
                    nc.scalar.activation(th[:, :, :w], cv, _TANH)
                    nc.vector.tensor_mul(h_b[:, :, :w], G[:, 6:8, :w],
                                         th[:, :, :w])

                    # filler matmuls keep the PE's HAM activity window busy
                    # through the latency-bound tail
                    for _d in range(6):
                        psd = psumf.tile([128, CH], F32, tag="psF")
                        nc.tensor.matmul(psd[:, :CH], wih[:, 0:128],
                                         wih[:, 0:CH],
                                         start=True, stop=True)

                    # projection for chunks that are now finished
                    def _alloc_d():
                        psj = psump.tile([128, CH], F32, tag="psP",
                                         name="psjd")
                        return psj[:]
                    for j in range(NCH):
                        if proj_at[j] == t:
                            emit_proj(j, _alloc_d)

    nc.compile()
    return nc


# ------------------------------------------------------------------ kernel

def _make_in_maps(pp, W_ih, W_hh, b_ih, b_hh, W_out):
    bf = mybir.dt.np(BF16)
    f8 = mybir.dt.np(FP8)
    wl = np.stack([
        np.ascontiguousarray(W_ih.T),          # [F=128, 4H]
        np.ascontiguousarray(W_hh.T[:128]),    # [128, 4H]
        np.ascontiguousarray(W_hh.T[128:]),    # [128, 4H]
    ]).astype(np.float32)
    # whh8[p, mi, k, m] = W_hh[mi*128+m, k*128+p]
    whh8 = W_hh.reshape(8, 128, 2, 128).transpose(3, 0, 2, 1)
    bc = np.ascontiguousarray((b_ih + b_hh).astype(np.float32).reshape(8, 128).T)
    be8 = np.zeros((8, 8, 64), np.float32)
    be8[np.arange(8), np.arange(8), :] = 1.0
    be4 = np.zeros((4, 4, 128), np.float32)
    be4[np.arange(4), np.arange(4), :] = 1.0
    wohb = np.stack([W_out[128:256], W_out[256:384]])
    maps = []
    for c in range(NCORES):
        m = {"xseq": pp["xseq"][c].astype(bf),
             "xown": pp["xown"][c].astype(np.float32),
             "wl": wl.astype(bf),
             "whh8": np.ascontiguousarray(whh8).astype(f8),
             "wox": np.ascontiguousarray(W_out[0:128]).astype(np.float32),
             "wohb": wohb.astype(bf),
             "bc": bc,
             "bct8": bc.T.astype(bf),
             "bcrep": np.broadcast_to(bc.T[None, :, :] / 128.0,
                                      (128, 8, 128)).astype(bf),
             "ones": np.ones((128, 512), np.float32).astype(bf),
             "be8": be8.astype(bf),
             "be4": be4.astype(bf)}
        maps.append(m)
    return maps


def run(inputs, trace=False, mm_dt=None):
    """Full pipeline; returns (output [N, OUT], BassKernelResults, pp)."""
    input_matrix = np.asarray(inputs["input_matrix"], np.float32)
    adjacency = np.asarray(inputs["adjacency"])
    W_ih = np.asarray(inputs["W_ih"], np.float32)
    W_hh = np.asarray(inputs["W_hh"], np.float32)
    b_ih = np.asarray(inputs["b_ih"], np.float32)
    b_hh = np.asarray(inputs["b_hh"], np.float32)
    W_out = np.asarray(inputs["W_out"], np.float32)

    use_dr = os.environ.get("KDR", "1") == "1"
    pp = _preprocess(input_matrix, adjacency)
    nc = build_program(pp["T"], pp["M"], pp["off"], pp["S"], pp["AC"], use_dr)
    in_maps = _make_in_maps(pp, W_ih, W_hh, b_ih, b_hh, W_out)
    res = run_bass_kernel_spmd(nc, in_maps, list(range(NCORES)), trace=trace)

    N = input_matrix.shape[0]
    out = np.zeros((N, OUT), np.float32)
    for c in range(NCORES):
        oc = np.asarray(res.results[c]["out"]).reshape(OUT, pp["AC"])
        cn = pp["col_node"][c]
        valid = cn >= 0
        out[cn[valid]] = oc[:, valid].T
        if len(pp["deg0"][c]):
            z = pp["deg0"][c]
            out[z] = input_matrix[z] @ W_out[:F]  # h = 0 for degree-0 nodes
    return out, res, pp


def kernel(**inputs) -> np.ndarray:
    out, _, _ = run(inputs, trace=False)
    return out
